# revision 1
# baseline (speedup 1.0000x reference)
"""Trainium2 Bass kernel for nn_CDFVarianceLoss.

Math (per sample b, per tensor z in {pred[b], target[b]}, N = 65536):
    z' = (z - min z) / (max z - min z + 1e-6)
    h_j = sum_n exp(-(z'_n - c_j)^2 / (2*sigma^2)) + 1e-6,  c_j = j/63, j < 64
    cdf = cumsum(h / sum_j h)
    loss = mean_{b,j} (cdf_pred[b,j] - cdf_target[b,j])^2

Distribution: data-parallel over the batch — 16 samples over 8 cores,
2 samples per core.  Each core returns the per-(sample, bin) squared CDF
difference [2, 64]; the host averages.

Per-core pipeline:
  - load z natural [128, 512] fp32; DVE per-partition min/max; the
    128-way cross-partition reduction is finished via PE transpose
    (stats -> [2,128] PSUM) + tiny DVE reduces, and the resulting
    (-zmin, 1/(zmax-zmin+eps)) scalars are broadcast back to all 128
    partitions with a ones-column matmul — no DRAM round trips.
  - DVE: z' = (z + (-zmin)) * s (fp32); z'^2 (fp32); then bf16 hi/lo
    splits z' = zhi+zlo, z'^2 = z2hi+z2lo (combined exact to ~2^-17 —
    needed because the exponent is amplified by alpha=200, and bf16
    matmuls stream 4x faster than fp32 on the PE)
  - DMA-reshape the bf16 splits into row tiles [10, CHUNK] whose rows are
    (zhi, zhi, zlo, z2hi, z2lo) x {pred, target}
  - PE bf16 matmul, static block-diag lhsT [10,128] with columns
    [m_hi, m_lo, m_hi, 1, 1] (m = -2c split hi/lo):
    q[j,n] = z'^2 - 2*c_j*z' accumulated exactly in fp32 PSUM
  - ACT: exp(-alpha*q - alpha*c_j^2) with static per-partition bias and
    accum_out -> per-bin partial sums (the only O(N*BINS) pass)
  - DVE reduce -> h; +eps; segmented bin-sums + reciprocal broadcast via
    two tiny matmuls with static 0/1 block matrices
  - PE matmul with static [128,64] cumsum-difference matrix -> cdf diff
  - ACT square -> DMA out
"""

import numpy as np

B = 16
N = 65536
BINS = 64
SIGMA = 0.05
EPS = 1e-6
ALPHA = 0.5 / SIGMA**2  # 200.0
NCORES = 8
SPC = B // NCORES  # samples per core
P = 128
F = N // P  # 512 natural free dim
CHUNK = 16384  # row-layout chunk (elements per rhs row tile)
NCHUNK = N // CHUNK  # 4
MMN = 512  # matmul moving free dim (one PSUM bank of fp32 output)
ACTB = 4  # matmuls per ACT block (PSUM tile = 4 banks)
K = 10  # rhs rows: 5 per tensor x 2 tensors

_CACHE = {}


def _np_bf16_split(x):
    import ml_dtypes

    hi = x.astype(ml_dtypes.bfloat16).astype(np.float32)
    lo = (x - hi).astype(ml_dtypes.bfloat16).astype(np.float32)
    return hi, lo


def _build_nc():
    import concourse.bass as bass
    import concourse.bacc as bacc
    import concourse.tile as tile
    import ml_dtypes
    from concourse import mybir
    from contextlib import ExitStack

    f32 = mybir.dt.float32
    bf16 = mybir.dt.bfloat16
    AX = mybir.AxisListType
    OP = mybir.AluOpType
    ACTF = mybir.ActivationFunctionType

    nc = bacc.Bacc()
    pred_d = nc.declare_dram_parameter("pred", [SPC, N], f32, isOutput=False)
    targ_d = nc.declare_dram_parameter("target", [SPC, N], f32, isOutput=False)
    out_d = nc.declare_dram_parameter("out_sq", [SPC, BINS], f32, isOutput=True)

    c = np.linspace(0.0, 1.0, BINS, dtype=np.float32)
    m_hi, m_lo = _np_bf16_split(-2.0 * c)
    coeffs = np.stack([m_hi, m_lo, m_hi, np.ones(BINS, np.float32),
                       np.ones(BINS, np.float32)])  # [5, 64]
    lhsT_main_np = np.zeros((K, P), np.float32)
    lhsT_main_np[0:5, :BINS] = coeffs
    lhsT_main_np[5:10, BINS:] = coeffs
    lhsT_main_np = lhsT_main_np.astype(ml_dtypes.bfloat16)
    bias_np = np.concatenate([-ALPHA * c * c, -ALPHA * c * c]).reshape(P, 1)
    bias_np = bias_np.astype(np.float32)
    # cumsum-and-subtract: out[m] = sum_{k<=m} hn_pred[k] - sum_{k<=m} hn_targ[k]
    lhsT_tail_np = np.zeros((P, BINS), np.float32)
    for mcol in range(BINS):
        lhsT_tail_np[: mcol + 1, mcol] = 1.0
        lhsT_tail_np[BINS : BINS + mcol + 1, mcol] = -1.0
    # segmented-sum / segmented-broadcast 0/1 blocks
    blk_np = np.zeros((P, 2), np.float32)
    blk_np[:BINS, 0] = 1.0
    blk_np[BINS:, 1] = 1.0
    ones_row_np = np.ones((1, P), np.float32)
    ident_np = np.eye(P, dtype=np.float32)

    lhsT_main_d = nc.inline_tensor(lhsT_main_np, name="lhsT_main")
    bias_d = nc.inline_tensor(bias_np, name="bias_col")
    lhsT_tail_d = nc.inline_tensor(lhsT_tail_np, name="lhsT_tail")
    blk_d = nc.inline_tensor(blk_np, name="blk")
    blkT_d = nc.inline_tensor(np.ascontiguousarray(blk_np.T), name="blkT")
    ones_d = nc.inline_tensor(ones_row_np, name="ones_row")
    ident_d = nc.inline_tensor(ident_np, name="ident")

    with tile.TileContext(nc) as tc, ExitStack() as ctx:
        singles = ctx.enter_context(tc.tile_pool(name="singles", bufs=1))
        nat = ctx.enter_context(tc.tile_pool(name="nat", bufs=2))
        norm = ctx.enter_context(tc.tile_pool(name="norm", bufs=2))
        small = ctx.enter_context(tc.tile_pool(name="small", bufs=2))
        rows = ctx.enter_context(tc.tile_pool(name="rows", bufs=3))
        scr = ctx.enter_context(tc.tile_pool(name="scr", bufs=2))
        hp = ctx.enter_context(tc.tile_pool(name="hp", bufs=2))
        ps_pool = ctx.enter_context(tc.tile_pool(name="ps", bufs=2, space="PSUM"))
        st_pool = ps_pool

        # DMA queue roles: sync carries the steady-state reshape stream;
        # gpsimd carries loads/consts/outputs so they never sit behind a
        # dependency-blocked reshape (in-order queues).
        def dma_ld(out, in_):
            nc.gpsimd.dma_start(out=out, in_=in_)

        def dma_rs(out, in_):
            nc.sync.dma_start(out=out, in_=in_)

        lhsT_main_sb = singles.tile([K, P], bf16)
        dma_ld(lhsT_main_sb, lhsT_main_d[:, :])
        bias_sb = singles.tile([P, 1], f32)
        dma_ld(bias_sb, bias_d[:, :])
        lhsT_tail_sb = singles.tile([P, BINS], f32)
        dma_ld(lhsT_tail_sb, lhsT_tail_d[:, :])
        blk_sb = singles.tile([P, 2], f32)
        dma_ld(blk_sb, blk_d[:, :])
        blkT_sb = singles.tile([2, P], f32)
        dma_ld(blkT_sb, blkT_d[:, :])
        ones_sb = singles.tile([1, P], f32)
        dma_ld(ones_sb, ones_d[:, :])
        ident_sb = singles.tile([P, P], f32)
        dma_ld(ident_sb, ident_d[:, :])

        def load_and_norm(p):
            zA = nat.tile([P, F], f32, tag="zA")
            dma_ld(zA, pred_d[p, :].rearrange("(p f) -> p f", p=P))
            zB = nat.tile([P, F], f32, tag="zB")
            dma_ld(zB, targ_d[p, :].rearrange("(p f) -> p f", p=P))

            def norm_one(z, tag):
                # per-partition (min, -max) over the free dim
                mm = small.tile([P, 2], f32, tag=f"mm{tag}")
                nc.vector.tensor_reduce(out=mm[:, 0:1], in_=z, axis=AX.X, op=OP.min)
                nc.vector.tensor_reduce(
                    out=mm[:, 1:2], in_=z, axis=AX.X, op=OP.max, negate=True
                )
                # finish the cross-partition reduction via PE transpose:
                # one min-reduce of [2,128] gives (zmin, -zmax)
                t1p = st_pool.tile([2, P], f32, tag="ps")
                nc.tensor.transpose(t1p, mm, ident_sb[:, :])
                t1 = small.tile([2, P], f32, tag=f"t1{tag}")
                nc.vector.tensor_copy(t1, t1p)
                mn2 = small.tile([2, 1], f32, tag=f"mn2{tag}")
                nc.vector.tensor_reduce(out=mn2, in_=t1, axis=AX.X, op=OP.min)
                # gather (zmin, -zmax) onto partition 0
                t2p = st_pool.tile([1, 2], f32, tag="ps")
                nc.tensor.transpose(t2p, mn2, ident_sb[0:2, 0:2])
                t2 = small.tile([1, 2], f32, tag=f"t2{tag}")
                nc.vector.tensor_copy(t2, t2p)
                # sc = [-zmin, 1/(zmax - zmin + eps)] on partition 0
                sc = small.tile([1, 2], f32, tag=f"sc{tag}")
                nc.vector.tensor_scalar_mul(sc[0:1, 0:1], t2[0:1, 0:1], -1.0)
                r = small.tile([1, 1], f32, tag=f"r{tag}")
                # r = -((-zmax) + zmin) + eps = zmax - zmin + eps
                nc.vector.tensor_scalar(
                    r, t2[0:1, 1:2], t2[0:1, 0:1], -1.0, OP.add, OP.mult
                )
                nc.vector.tensor_scalar_add(r, r, EPS)
                nc.vector.reciprocal(sc[0:1, 1:2], r)
                # broadcast to all partitions with a ones-column matmul
                nbp = st_pool.tile([P, 2], f32, tag="ps")
                nc.tensor.matmul(nbp, ones_sb[:, :], sc, start=True, stop=True)
                nb = small.tile([P, 2], f32, tag=f"nb{tag}")
                nc.vector.tensor_copy(nb, nbp)
                zp = norm.tile([P, F], f32, tag=f"zp{tag}")
                nc.vector.tensor_scalar(zp, z, nb[:, 0:1], nb[:, 1:2], OP.add, OP.mult)
                zp2 = norm.tile([P, F], f32, tag=f"zp2{tag}")
                nc.vector.tensor_mul(zp2, zp, zp)
                # bf16 hi/lo splits (combined exact to ~2^-17)
                zhi = norm.tile([P, F], bf16, tag=f"zhi{tag}")
                nc.vector.tensor_copy(zhi, zp)
                zlo = norm.tile([P, F], bf16, tag=f"zlo{tag}")
                nc.vector.tensor_sub(zlo, zp, zhi)
                z2hi = norm.tile([P, F], bf16, tag=f"z2hi{tag}")
                nc.vector.tensor_copy(z2hi, zp2)
                z2lo = norm.tile([P, F], bf16, tag=f"z2lo{tag}")
                nc.vector.tensor_sub(z2lo, zp2, z2hi)
                return zhi, zlo, z2hi, z2lo

            rowsA = norm_one(zA, "A")
            rowsB = norm_one(zB, "B")
            # rhs row order must match lhsT_main rows
            return [rowsA[0], rowsA[0], rowsA[1], rowsA[2], rowsA[3],
                    rowsB[0], rowsB[0], rowsB[1], rowsB[2], rowsB[3]]

        mm_per_chunk = CHUNK // MMN  # 32
        blocks = []  # list of (start_mm, n_mm) per ACT block
        i = 0
        while i < mm_per_chunk:
            n = min(ACTB, mm_per_chunk - i)
            blocks.append((i, n))
            i += n
        pp = CHUNK // F  # natural partitions per chunk (32)

        srcs_p = [load_and_norm(p) for p in range(SPC)]
        hparts_p = []
        for p in range(SPC):
            hparts_t = hp.tile(
                [P, NCHUNK * len(blocks)], f32, tag=f"hparts{p}", name=f"hparts{p}"
            )
            hparts_p.append(hparts_t)
        # interleave the two samples' chunk pipelines so the ACT stream
        # stays dense across the whole kernel (no pair-boundary stall)
        for ch in range(NCHUNK):
            for p in range(SPC):
                srcs = srcs_p[p]
                hparts = hparts_p[p]
                rt = rows.tile([K, CHUNK], bf16, tag="rt")
                sl = slice(ch * pp, (ch + 1) * pp)
                for r, src in enumerate(srcs):
                    dma_rs(rt[r : r + 1, :], src[sl, :])
                for bi, (mm0, nmm) in enumerate(blocks):
                    ps = ps_pool.tile([P, ACTB * MMN], f32, tag="ps")
                    for k in range(nmm):
                        col = (mm0 + k) * MMN
                        nc.tensor.matmul(
                            ps[:, k * MMN : (k + 1) * MMN],
                            lhsT_main_sb[:, :],
                            rt[:, col : col + MMN],
                            start=True,
                            stop=True,
                        )
                    sc_t = scr.tile([P, ACTB * MMN], f32, tag="sc")
                    icol = ch * len(blocks) + bi
                    nc.scalar.activation(
                        out=sc_t[:, : nmm * MMN],
                        in_=ps[:, : nmm * MMN],
                        func=ACTF.Exp,
                        bias=bias_sb[:, 0:1],
                        scale=-ALPHA,
                        accum_out=hparts[:, icol : icol + 1],
                    )

        for p in range(SPC):
            hparts = hparts_p[p]
            hcol = small.tile([P, 1], f32, tag="hcol")
            nc.vector.tensor_reduce(out=hcol, in_=hparts, axis=AX.X, op=OP.add)
            heps = small.tile([P, 1], f32, tag="heps")
            nc.vector.tensor_scalar_add(heps, hcol, EPS)
            # segmented sums over the two 64-bin halves via 0/1 matmul,
            # reciprocal, then segmented broadcast via the transposed block
            s2p = st_pool.tile([2, 1], f32, tag="ps")
            nc.tensor.matmul(s2p, blk_sb[:, :], heps, start=True, stop=True)
            sinv2 = small.tile([2, 1], f32, tag="sinv2")
            nc.vector.reciprocal(sinv2, s2p)
            sbp = st_pool.tile([P, 1], f32, tag="ps")
            nc.tensor.matmul(sbp, blkT_sb[:, :], sinv2, start=True, stop=True)
            sinv = small.tile([P, 1], f32, tag="sinv")
            nc.vector.tensor_copy(sinv, sbp)
            hn = small.tile([P, 1], f32, tag="hn")
            nc.vector.tensor_mul(hn, heps, sinv)
            pst = st_pool.tile([BINS, 1], f32, tag="ps")
            nc.tensor.matmul(pst, lhsT_tail_sb[:, :], hn, start=True, stop=True)
            sq = small.tile([BINS, 1], f32, tag="sq")
            nc.scalar.square(sq, pst)
            dma_ld(out_d[p, :], sq[:, 0:1])

    nc.compile()
    return nc


def kernel(pred: np.ndarray, target: np.ndarray) -> np.ndarray:
    from concourse.bass_utils import run_bass_kernel_spmd

    if "nc" not in _CACHE:
        _CACHE["nc"] = _build_nc()
    nc = _CACHE["nc"]

    pred = np.ascontiguousarray(np.asarray(pred, np.float32).reshape(B, N))
    target = np.ascontiguousarray(np.asarray(target, np.float32).reshape(B, N))
    in_maps = [
        {
            "pred": pred[i * SPC : (i + 1) * SPC],
            "target": target[i * SPC : (i + 1) * SPC],
        }
        for i in range(NCORES)
    ]
    res = run_bass_kernel_spmd(nc, in_maps, list(range(NCORES)))
    sq = np.concatenate([r["out_sq"] for r in res.results], axis=0)  # [16, 64]
    return np.float32(np.mean(sq, dtype=np.float64))



# revision 3
# speedup vs baseline: 1.2133x; 1.2133x over previous
"""Trainium2 Bass kernel for nn_CDFVarianceLoss.

Math (per sample b, per tensor z in {pred[b], target[b]}, N = 65536):
    z' = (z - min z) / (max z - min z + 1e-6)
    h_j = sum_n exp(-(z'_n - c_j)^2 / (2*sigma^2)) + 1e-6,  c_j = j/63, j < 64
    cdf = cumsum(h / sum_j h)
    loss = mean_{b,j} (cdf_pred[b,j] - cdf_target[b,j])^2

Key algorithmic move: the 64-bin soft histogram is a Gaussian KDE, so the
device only samples the KDE u_m = sum_n exp(-alpha (z'_n - y_m)^2) on a
coarse M=16 grid y (the KDE is sigma-smooth, so a fixed least-squares
interpolation matrix A reconstructs all 64 bins: h ~= A @ u, verified to
give ~1e-6 relative loss error offline).  This cuts the O(N*BINS) ACT
exp work 4x vs the dense 64-bin evaluation.

Distribution: data-parallel over the batch (2 samples/core).  Each core
packs 8 "slots" = 4 (sample, tensor) units x 2 column halves into the 128
PSUM partitions (slot s -> partitions 16s..16s+16, its own 5 bf16 feature
rows in the [40, 128] block-diagonal static lhsT).  Each matmul column
carries 8 elements (one per slot), so the whole core needs only 32768
matmul columns / ACT-exp columns.

Per-core pipeline:
  - load z natural [128, 512] fp32; DVE per-partition (-min, max); GPSIMD
    partition_all_reduce(max) finishes the cross-partition reduction in
    one op ([-gmin, gmax] on every partition); tiny DVE ops give the
    (-zmin, 1/(zmax-zmin+eps)) per-partition scalars.
  - z' = (z + (-zmin)) * s (DVE fp32); z'^2 (ACT square); bf16 hi/lo
    splits z' = zhi+zlo (ACT copy + DVE sub), z'^2 = z2hi+z2lo (GPSIMD
    copy + sub) -- combined exact to ~2^-17, needed because alpha=200
    amplifies the exponent and bf16 matmuls stream 4x faster on the PE.
  - DMA-reshape the splits into row tiles [40, CHUNK]: slot s rows
    (zhi, zhi, zlo, z2hi, z2lo) of its unit/half.
  - PE bf16 matmul, static block-diag lhsT [40, 128] with per-slot coeff
    columns [m_hi, m_lo, m_hi, 1, 1] (m = -2y split hi/lo):
    q[16s+m, n] = z'^2 - 2 y_m z' in fp32 PSUM.
  - ACT exp(-alpha q - alpha y_m^2) with static bias, accum_out -> u
    partial sums (the only O(N*M) pass).
  - DVE reduce -> u [128]; DMA out.
Host: u -> h = A u + eps -> normalize/cumsum/diff/square/mean in fp64
(A is a fixed 64x16 matrix; trivial flops, same spirit as the host mean).
"""

import numpy as np

B = 16
N = 65536
BINS = 64
SIGMA = 0.05
EPS = 1e-6
ALPHA = 0.5 / SIGMA**2  # 200.0
NCORES = 8
SPC = B // NCORES  # samples per core
P = 128
F = N // P  # 512 natural free dim
M = 16  # KDE grid points per (sample, tensor) unit
RPAD = 0.05  # grid span padding beyond [0, 1]
NSLOT = 8  # 4 units x 2 column halves
KROWS = 5 * NSLOT  # 40 rhs rows
COLS = N // 2  # matmul columns per core (each col = 8 elements)
CHUNK = 8192  # columns per reshape tile
NCHUNK = COLS // CHUNK  # 4
MMN = 512  # matmul moving free dim
ACTB = 4  # matmuls per ACT block (PSUM tile = 4 banks)

_CACHE = {}


def _np_bf16_split(x):
    import ml_dtypes

    hi = x.astype(ml_dtypes.bfloat16).astype(np.float32)
    lo = (x - hi).astype(ml_dtypes.bfloat16).astype(np.float32)
    return hi, lo


def _grid():
    return np.linspace(-RPAD, 1.0 + RPAD, M)


def _interp_matrix():
    """Least-squares fit: h(c_j) ~= sum_m A[j,m] u(y_m) for any empirical
    distribution of z in [0,1] (exact for the KDE's sigma-limited spectrum)."""
    yg = _grid()
    c = np.linspace(0.0, 1.0, BINS)
    zt = np.linspace(0.0, 1.0, 8001)
    Bm = np.exp(-ALPHA * (zt[:, None] - yg[None, :]) ** 2)  # [T, M]
    G = np.exp(-ALPHA * (c[:, None] - zt[None, :]) ** 2)  # [BINS, T]
    A = np.linalg.solve(Bm.T @ Bm + 1e-9 * np.eye(M), Bm.T @ G.T).T
    return A  # [BINS, M] float64


def _build_nc():
    import concourse.bass as bass
    import concourse.bacc as bacc
    import concourse.tile as tile
    import ml_dtypes
    from concourse import mybir
    from concourse import bass_isa
    from contextlib import ExitStack

    f32 = mybir.dt.float32
    bf16 = mybir.dt.bfloat16
    AX = mybir.AxisListType
    OP = mybir.AluOpType
    ACTF = mybir.ActivationFunctionType

    nc = bacc.Bacc()
    pred_d = nc.declare_dram_parameter("pred", [SPC, N], f32, isOutput=False)
    targ_d = nc.declare_dram_parameter("target", [SPC, N], f32, isOutput=False)
    out_d = nc.declare_dram_parameter("u_out", [1, P], f32, isOutput=True)

    yg = _grid().astype(np.float32)
    m_hi, m_lo = _np_bf16_split(-2.0 * yg)
    coeffs = np.stack([m_hi, m_lo, m_hi, np.ones(M, np.float32),
                       np.ones(M, np.float32)])  # [5, M]
    lhsT_np = np.zeros((KROWS, P), np.float32)
    for s in range(NSLOT):
        lhsT_np[5 * s : 5 * s + 5, M * s : M * s + M] = coeffs
    lhsT_np = lhsT_np.astype(ml_dtypes.bfloat16)
    bias_np = np.tile(-ALPHA * yg * yg, NSLOT).reshape(P, 1).astype(np.float32)

    lhsT_d = nc.inline_tensor(lhsT_np, name="lhsT_main")
    bias_d = nc.inline_tensor(bias_np, name="bias_col")

    with tile.TileContext(nc) as tc, ExitStack() as ctx:
        singles = ctx.enter_context(tc.tile_pool(name="singles", bufs=1))
        nat = ctx.enter_context(tc.tile_pool(name="nat", bufs=2))
        scr = ctx.enter_context(tc.tile_pool(name="scr", bufs=2))
        small = ctx.enter_context(tc.tile_pool(name="small", bufs=2))
        splits = ctx.enter_context(tc.tile_pool(name="splits", bufs=1))
        rows = ctx.enter_context(tc.tile_pool(name="rows", bufs=3))
        scs = ctx.enter_context(tc.tile_pool(name="scs", bufs=2))
        hp = ctx.enter_context(tc.tile_pool(name="hp", bufs=1))
        ps_pool = ctx.enter_context(tc.tile_pool(name="ps", bufs=2, space="PSUM"))

        # DMA queue roles: sync carries input loads then the steady-state
        # reshape stream (loads are enqueued first so the in-order queue
        # drains them before any dependency-blocked reshape); gpsimd
        # carries consts and the single output.
        lhsT_sb = singles.tile([KROWS, P], bf16)
        nc.gpsimd.dma_start(out=lhsT_sb, in_=lhsT_d[:, :])
        bias_sb = singles.tile([P, 1], f32)
        nc.gpsimd.dma_start(out=bias_sb, in_=bias_d[:, :])

        unit_rows = []
        for u in range(4):
            p, t = divmod(u, 2)
            src_d = pred_d if t == 0 else targ_d
            z = nat.tile([P, F], f32, tag=f"z{u}")
            nc.sync.dma_start(out=z, in_=src_d[p, :].rearrange("(p f) -> p f", p=P))
            # cross-partition min/max: per-partition (-min, max) on DVE,
            # then one GPSIMD all-reduce(max) -> [-gmin, gmax] everywhere
            mm = small.tile([P, 2], f32, tag=f"mm{u}")
            nc.vector.tensor_reduce(
                out=mm[:, 0:1], in_=z, axis=AX.X, op=OP.min, negate=True
            )
            nc.vector.tensor_reduce(out=mm[:, 1:2], in_=z, axis=AX.X, op=OP.max)
            mmr = small.tile([P, 2], f32, tag=f"mmr{u}")
            nc.gpsimd.partition_all_reduce(mmr, mm, P, bass_isa.ReduceOp.max)
            r = small.tile([P, 1], f32, tag=f"r{u}")
            # r = (gmax + (-gmin)) + eps
            nc.vector.tensor_scalar(r, mmr[:, 1:2], mmr[:, 0:1], EPS, OP.add, OP.add)
            s = small.tile([P, 1], f32, tag=f"s{u}")
            nc.vector.reciprocal(s, r)
            zp = scr.tile([P, F], f32, tag=f"zp{u}")
            nc.vector.tensor_scalar(zp, z, mmr[:, 0:1], s, OP.add, OP.mult)
            zp2 = scr.tile([P, F], f32, tag=f"zp2{u}")
            nc.scalar.square(zp2, zp)
            zhi = splits.tile([P, F], bf16, tag=f"zhi{u}")
            nc.scalar.copy(zhi, zp)
            zlo = splits.tile([P, F], bf16, tag=f"zlo{u}")
            nc.vector.tensor_sub(zlo, zp, zhi)
            z2hi = splits.tile([P, F], bf16, tag=f"z2hi{u}")
            nc.gpsimd.tensor_copy(z2hi, zp2)
            z2lo = splits.tile([P, F], bf16, tag=f"z2lo{u}")
            nc.gpsimd.tensor_sub(z2lo, zp2, z2hi)
            unit_rows.append((zhi, zhi, zlo, z2hi, z2lo))

        hparts = hp.tile([P, NCHUNK * (CHUNK // (ACTB * MMN))], f32, name="hparts")
        pp = CHUNK // F  # natural partitions per chunk slice (16)
        for ch in range(NCHUNK):
            rt = rows.tile([KROWS, CHUNK], bf16, tag="rt")
            for s8 in range(NSLOT):
                u, half = divmod(s8, 2)
                p0 = 64 * half + pp * ch
                for k, src in enumerate(unit_rows[u]):
                    nc.sync.dma_start(
                        out=rt[5 * s8 + k : 5 * s8 + k + 1, :],
                        in_=src[p0 : p0 + pp, :],
                    )
            for hb in range(CHUNK // (ACTB * MMN)):
                ps = ps_pool.tile([P, ACTB * MMN], f32, tag="ps")
                for i in range(ACTB):
                    c0 = hb * ACTB * MMN + i * MMN
                    nc.tensor.matmul(
                        ps[:, i * MMN : (i + 1) * MMN],
                        lhsT_sb[:, :],
                        rt[:, c0 : c0 + MMN],
                        start=True,
                        stop=True,
                    )
                sc_t = scs.tile([P, ACTB * MMN], bf16, tag="sc")
                icol = ch * (CHUNK // (ACTB * MMN)) + hb
                nc.scalar.activation(
                    out=sc_t,
                    in_=ps,
                    func=ACTF.Exp,
                    bias=bias_sb[:, 0:1],
                    scale=-ALPHA,
                    accum_out=hparts[:, icol : icol + 1],
                )

        uvec = small.tile([P, 1], f32, tag="uvec")
        nc.vector.tensor_reduce(out=uvec, in_=hparts, axis=AX.X, op=OP.add)
        nc.gpsimd.dma_start(out=out_d[0, :], in_=uvec[:, 0:1])

    nc.compile()
    return nc


def kernel(pred: np.ndarray, target: np.ndarray) -> np.ndarray:
    from concourse.bass_utils import run_bass_kernel_spmd

    if "nc" not in _CACHE:
        _CACHE["nc"] = _build_nc()
        _CACHE["A"] = _interp_matrix()
    nc = _CACHE["nc"]
    A = _CACHE["A"]

    pred = np.ascontiguousarray(np.asarray(pred, np.float32).reshape(B, N))
    target = np.ascontiguousarray(np.asarray(target, np.float32).reshape(B, N))
    in_maps = [
        {
            "pred": pred[i * SPC : (i + 1) * SPC],
            "target": target[i * SPC : (i + 1) * SPC],
        }
        for i in range(NCORES)
    ]
    res = run_bass_kernel_spmd(nc, in_maps, list(range(NCORES)))
    us = np.stack([r["u_out"][0] for r in res.results], axis=0)  # [8, 128]
    us = us.astype(np.float64).reshape(NCORES, 4, 2, M)  # [core, unit, half, M]
    u = us.sum(axis=2).reshape(NCORES * 2, 2, M)  # [16 samples, pred/targ, M]
    h = u @ A.T + EPS  # [16, 2, 64]
    cdf = np.cumsum(h / h.sum(axis=-1, keepdims=True), axis=-1)
    return np.float32(np.mean((cdf[:, 0] - cdf[:, 1]) ** 2))


# revision 9
# speedup vs baseline: 2.3606x; 1.9456x over previous
"""Trainium2 Bass kernel for nn_CDFVarianceLoss.

Math (per sample b, per tensor z in {pred[b], target[b]}, N = 65536):
    z' = (z - min z) / (max z - min z + 1e-6)
    h_j = sum_n exp(-(z'_n - c_j)^2 / (2*sigma^2)) + 1e-6,  c_j = j/63, j < 64
    cdf = cumsum(h / sum_j h)
    loss = mean_{b,j} (cdf_pred[b,j] - cdf_target[b,j])^2

Algorithmic core: the 64-bin soft histogram is a Gaussian KDE, so the
device only samples the KDE u_m = sum_n exp(-alpha (z'_n - y_m)^2) on a
coarse M=16 grid y (the KDE spectrum is sigma-limited, so a fixed
least-squares matrix A reconstructs all 64 bins: h ~= A u, ~1e-6 relative
loss error verified offline; bf16-quantized z adds ~7e-5).  This cuts the
O(N*BINS) ACT work 4x vs dense 64-bin evaluation.

Evaluation trick: Derivative_Erf(x) = 2/sqrt(pi) exp(-x^2), so the whole
Gaussian evaluation needs only a LINEAR PSUM input z (no z^2 / no bf16
hi-lo splits): ACT computes DErf(scale_p * z + bias_p) where the
per-partition scale_p = k s_u and bias_p = -k (s_u zmin_u + y_m), k =
sqrt(alpha), fold the minmax normalization in for free (the 2/sqrt(pi)
constant cancels in the cdf normalization).  The PE matmul is a one-hot
[8, 128] bf16 broadcast: partition 16 s + m holds raw z of slot s, so
each matmul column carries 8 elements (one per slot = (sample, tensor,
column-half)) and the core needs only 32768 ACT/PE columns total.

Distribution: data-parallel over the batch (2 samples/core), 8 cores;
device returns the raw KDE samples u [128] per core; the host applies A,
eps, normalize/cumsum/diff/square/mean in fp64 (A is 64x16; trivial
flops, same spirit as the baseline's host-side mean).
"""

import numpy as np

B = 16
N = 65536
BINS = 64
SIGMA = 0.05
EPS = 1e-6
ALPHA = 0.5 / SIGMA**2  # 200.0
KSC = float(np.sqrt(ALPHA))  # DErf arg scale
NCORES = 8
SPC = B // NCORES  # samples per core
P = 128
F = N // P  # 512 natural free dim
M = 16  # KDE grid points per (sample, tensor) unit
RPAD = 0.05  # grid span padding beyond [0, 1]
NSLOT = 8  # 4 units x 2 column halves
COLS = N // 2  # matmul columns per core (each col = 8 elements)
CHUNK = 8192  # columns per reshape tile
NCHUNK = COLS // CHUNK  # 4
MMN = 512  # matmul moving free dim
ACTB = 4  # matmuls per ACT block (PSUM tile = 4 banks)

_CACHE = {}


def _grid():
    return np.linspace(-RPAD, 1.0 + RPAD, M)


def _interp_matrix():
    """Least-squares fit: h(c_j) ~= sum_m A[j,m] u(y_m) for any empirical
    distribution of z in [0,1] (the KDE's spectrum is sigma-limited, so the
    coarse grid determines it; verified offline to ~1e-6 loss error)."""
    yg = _grid()
    c = np.linspace(0.0, 1.0, BINS)
    zt = np.linspace(0.0, 1.0, 8001)
    Bm = np.exp(-ALPHA * (zt[:, None] - yg[None, :]) ** 2)  # [T, M]
    G = np.exp(-ALPHA * (c[:, None] - zt[None, :]) ** 2)  # [BINS, T]
    A = np.linalg.solve(Bm.T @ Bm + 1e-9 * np.eye(M), Bm.T @ G.T).T
    return A  # [BINS, M] float64


def _build_nc():
    import concourse.bass as bass
    import concourse.bacc as bacc
    import concourse.tile as tile
    import ml_dtypes
    from concourse import mybir
    from concourse import bass_isa
    from contextlib import ExitStack

    f32 = mybir.dt.float32
    bf16 = mybir.dt.bfloat16
    AX = mybir.AxisListType
    OP = mybir.AluOpType
    ACTF = mybir.ActivationFunctionType

    nc = bacc.Bacc()
    pred_d = nc.declare_dram_parameter("pred", [SPC, N], f32, isOutput=False)
    targ_d = nc.declare_dram_parameter("target", [SPC, N], f32, isOutput=False)
    out_d = nc.declare_dram_parameter("u_out", [1, P], f32, isOutput=True)

    yg = _grid().astype(np.float32)
    # one-hot broadcast: slot s's z row -> partitions 16s..16s+16
    lhsT_np = np.zeros((NSLOT, P), np.float32)
    for s in range(NSLOT):
        lhsT_np[s, M * s : M * s + M] = 1.0
    lhsT_np = lhsT_np.astype(ml_dtypes.bfloat16)
    # static -k*y_m term of the bias, tiled per slot
    ky_np = np.tile(KSC * yg, NSLOT).reshape(P, 1).astype(np.float32)

    lhsT_d = nc.inline_tensor(lhsT_np, name="lhsT_main")
    ky_d = nc.inline_tensor(ky_np, name="ky_col")

    with tile.TileContext(nc) as tc, ExitStack() as ctx:
        singles = ctx.enter_context(tc.tile_pool(name="singles", bufs=1))
        nat = ctx.enter_context(tc.tile_pool(name="nat", bufs=2))
        small = ctx.enter_context(tc.tile_pool(name="small", bufs=2))
        splits = ctx.enter_context(tc.tile_pool(name="splits", bufs=1))
        rows = ctx.enter_context(tc.tile_pool(name="rows", bufs=3))
        scs = ctx.enter_context(tc.tile_pool(name="scs", bufs=2))
        hp = ctx.enter_context(tc.tile_pool(name="hp", bufs=1))
        ps_pool = ctx.enter_context(tc.tile_pool(name="ps", bufs=2, space="PSUM"))

        # queue roles: sync = input loads then reshape stream (in-order, loads
        # enqueued first); gpsimd = consts + final output (SWDGE, separate
        # resource from the global HWDGE the sync-queue DMAs serialize on).
        lhsT_sb = singles.tile([NSLOT, P], bf16)
        nc.sync.dma_start(out=lhsT_sb, in_=lhsT_d[:, :])
        ky_sb = singles.tile([P, 1], f32)
        nc.sync.dma_start(out=ky_sb, in_=ky_d[:, :])

        scale_col = hp.tile([P, 1], f32, name="scale_col")
        bias_col = hp.tile([P, 1], f32, name="bias_col")

        zbs = []
        for u in range(4):
            p, t = divmod(u, 2)
            src_d = pred_d if t == 0 else targ_d
            z = nat.tile([P, F], f32, tag=f"z{u}")
            nc.sync.dma_start(out=z, in_=src_d[p, :].rearrange("(p f) -> p f", p=P))
            # raw z in bf16 feeds the PE (quantization verified offline);
            # independent of the stats chain, so the reshape starts early
            zb = splits.tile([P, F], bf16, tag=f"zb{u}")
            nc.gpsimd.tensor_copy(zb, z)
            zbs.append(zb)
            # cross-partition min/max: per-partition (-min, max) on DVE, one
            # GPSIMD all-reduce(max) -> [-gmin, gmax] on every partition
            mm = small.tile([P, 2], f32, tag=f"mm{u}")
            nc.vector.tensor_reduce(
                out=mm[:, 0:1], in_=z, axis=AX.X, op=OP.min, negate=True
            )
            nc.vector.tensor_reduce(out=mm[:, 1:2], in_=z, axis=AX.X, op=OP.max)
            mmr = small.tile([P, 2], f32, tag=f"mmr{u}")
            nc.gpsimd.partition_all_reduce(mmr, mm, P, bass_isa.ReduceOp.max)
            # s = 1/(gmax - gmin + eps); ACT arg = (k s) z - k (s gmin + y_m)
            r = small.tile([P, 1], f32, tag=f"r{u}")
            nc.vector.tensor_scalar(r, mmr[:, 1:2], mmr[:, 0:1], EPS, OP.add, OP.add)
            s = small.tile([P, 1], f32, tag=f"s{u}")
            nc.vector.reciprocal(s, r)
            blk = slice(32 * u, 32 * u + 32)
            nc.vector.tensor_scalar_mul(scale_col[blk, :], s[blk, :], KSC)
            # t1 = s * (-gmin); bias = k*t1 - k*y_m
            t1 = small.tile([P, 1], f32, tag=f"t1{u}")
            nc.vector.tensor_mul(t1[blk, :], s[blk, :], mmr[blk, 0:1])
            nc.vector.tensor_scalar(
                bias_col[blk, :], t1[blk, :], KSC, ky_sb[blk, 0:1], OP.mult, OP.subtract
            )

        nblk = CHUNK // (ACTB * MMN)  # ACT blocks per chunk
        hparts = hp.tile([P, NCHUNK * nblk], f32, name="hparts")
        pp = CHUNK // F  # natural partitions per (chunk, half) slice (16)
        for ch in range(NCHUNK):
            rt = rows.tile([NSLOT, CHUNK], bf16, tag="rt")
            for u in range(4):
                for h in range(2):
                    p0 = 64 * h + pp * ch
                    nc.sync.dma_start(
                        out=rt[2 * u + h : 2 * u + h + 1, :],
                        in_=zbs[u][p0 : p0 + pp, :],
                    )
            for hb in range(nblk):
                ps = ps_pool.tile([P, ACTB * MMN], f32, tag="ps")
                for i in range(ACTB):
                    c0 = hb * ACTB * MMN + i * MMN
                    nc.tensor.matmul(
                        ps[:, i * MMN : (i + 1) * MMN],
                        lhsT_sb[:, :],
                        rt[:, c0 : c0 + MMN],
                        start=True,
                        stop=True,
                    )
                icol = ch * nblk + hb
                sc_t = scs.tile([P, ACTB * MMN], bf16, tag="sc")
                nc.scalar.activation(
                    out=sc_t,
                    in_=ps,
                    func=ACTF.Derivative_Erf,
                    bias=bias_col[:, 0:1],
                    scale=scale_col[:, 0:1],
                    accum_out=hparts[:, icol : icol + 1],
                )

        uvec = small.tile([P, 1], f32, tag="uvec")
        nc.vector.tensor_reduce(out=uvec, in_=hparts, axis=AX.X, op=OP.add)
        nc.sync.dma_start(out=out_d[0, :], in_=uvec[:, 0:1])

    nc.compile()
    return nc


def kernel(pred: np.ndarray, target: np.ndarray) -> np.ndarray:
    from concourse.bass_utils import run_bass_kernel_spmd

    if "nc" not in _CACHE:
        _CACHE["nc"] = _build_nc()
        _CACHE["A"] = _interp_matrix()
    nc = _CACHE["nc"]
    A = _CACHE["A"]

    pred = np.ascontiguousarray(np.asarray(pred, np.float32).reshape(B, N))
    target = np.ascontiguousarray(np.asarray(target, np.float32).reshape(B, N))
    in_maps = [
        {
            "pred": pred[i * SPC : (i + 1) * SPC],
            "target": target[i * SPC : (i + 1) * SPC],
        }
        for i in range(NCORES)
    ]
    res = run_bass_kernel_spmd(nc, in_maps, list(range(NCORES)))
    us = np.stack([r["u_out"][0] for r in res.results], axis=0)  # [8, 128]
    us = us.astype(np.float64).reshape(NCORES, 4, 2, M)  # [core, unit, half, M]
    u = us.sum(axis=2).reshape(NCORES * 2, 2, M)  # [16 samples, pred/targ, M]
    # the DErf 2/sqrt(pi) constant cancels in the cdf normalization
    h = u @ A.T + EPS
    cdf = np.cumsum(h / h.sum(axis=-1, keepdims=True), axis=-1)
    return np.float32(np.mean((cdf[:, 0] - cdf[:, 1]) ** 2))


# revision 13
# speedup vs baseline: 2.5831x; 1.0942x over previous
"""Trainium2 Bass kernel for nn_CDFVarianceLoss.

Math (per sample b, per tensor z in {pred[b], target[b]}, N = 65536):
    z' = (z - min z) / (max z - min z + 1e-6)
    h_j = sum_n exp(-(z'_n - c_j)^2 / (2*sigma^2)) + 1e-6,  c_j = j/63, j < 64
    cdf = cumsum(h / sum_j h)
    loss = mean_{b,j} (cdf_pred[b,j] - cdf_target[b,j])^2

Algorithmic core: the 64-bin soft histogram is a Gaussian KDE, so the
device only samples the KDE u_m = sum_n exp(-alpha (z'_n - y_m)^2) on a
coarse M=16 grid y (the KDE spectrum is sigma-limited, so a fixed
least-squares matrix A reconstructs all 64 bins: h ~= A u, ~1e-6 relative
loss error verified offline; bf16-quantized z adds ~7e-5).  This cuts the
O(N*BINS) ACT work 4x vs dense 64-bin evaluation.

Evaluation trick: Derivative_Erf(x) = 2/sqrt(pi) exp(-x^2), so the whole
Gaussian evaluation needs only a LINEAR PSUM input z (no z^2 / no bf16
hi-lo splits): ACT computes DErf(scale_p * z + bias_p) where the
per-partition scale_p = k s_u and bias_p = -k (s_u zmin_u + y_m), k =
sqrt(alpha), fold the minmax normalization in for free (the 2/sqrt(pi)
constant cancels in the cdf normalization).  The PE matmul is a one-hot
[8, 128] bf16 broadcast: partition 16 s + m holds raw z of slot s, so
each matmul column carries 8 elements (one per slot = (sample, tensor,
column-half)) and the core needs only 32768 ACT/PE columns total.

Distribution: data-parallel over the batch (2 samples/core), 8 cores;
device returns the raw KDE samples u [128] per core; the host applies A,
eps, normalize/cumsum/diff/square/mean in fp64 (A is 64x16; trivial
flops, same spirit as the baseline's host-side mean).
"""

import numpy as np

B = 16
N = 65536
BINS = 64
SIGMA = 0.05
EPS = 1e-6
ALPHA = 0.5 / SIGMA**2  # 200.0
KSC = float(np.sqrt(ALPHA))  # DErf arg scale
NCORES = 8
SPC = B // NCORES  # samples per core
P = 128
F = N // P  # 512 natural free dim
M = 16  # KDE grid points per (sample, tensor) unit
RPAD = 0.05  # grid span padding beyond [0, 1]
NSLOT = 8  # 4 units x 2 column halves
COLS = N // 2  # matmul columns per core (each col = 8 elements)
CHUNK = 8192  # columns per reshape tile
NCHUNK = COLS // CHUNK  # 4
MMN = 512  # matmul moving free dim
ACTB = 4  # matmuls per ACT block (PSUM tile = 4 banks)

_CACHE = {}


def _grid():
    return np.linspace(-RPAD, 1.0 + RPAD, M)


def _interp_matrix():
    """Least-squares fit: h(c_j) ~= sum_m A[j,m] u(y_m) for any empirical
    distribution of z in [0,1] (the KDE's spectrum is sigma-limited, so the
    coarse grid determines it; verified offline to ~1e-6 loss error)."""
    yg = _grid()
    c = np.linspace(0.0, 1.0, BINS)
    zt = np.linspace(0.0, 1.0, 8001)
    Bm = np.exp(-ALPHA * (zt[:, None] - yg[None, :]) ** 2)  # [T, M]
    G = np.exp(-ALPHA * (c[:, None] - zt[None, :]) ** 2)  # [BINS, T]
    A = np.linalg.solve(Bm.T @ Bm + 1e-9 * np.eye(M), Bm.T @ G.T).T
    return A  # [BINS, M] float64


def _build_nc():
    import concourse.bass as bass
    import concourse.bacc as bacc
    import concourse.tile as tile
    import ml_dtypes
    from concourse import mybir
    from concourse import bass_isa
    from contextlib import ExitStack

    f32 = mybir.dt.float32
    bf16 = mybir.dt.bfloat16
    AX = mybir.AxisListType
    OP = mybir.AluOpType
    ACTF = mybir.ActivationFunctionType

    nc = bacc.Bacc()
    pred_d = nc.declare_dram_parameter("pred", [SPC, N], f32, isOutput=False)
    targ_d = nc.declare_dram_parameter("target", [SPC, N], f32, isOutput=False)
    out_d = nc.declare_dram_parameter("u_out", [1, P], f32, isOutput=True)

    yg = _grid().astype(np.float32)
    # one-hot broadcast: slot s's z row -> partitions 16s..16s+16
    lhsT_np = np.zeros((NSLOT, P), np.float32)
    for s in range(NSLOT):
        lhsT_np[s, M * s : M * s + M] = 1.0
    lhsT_np = lhsT_np.astype(ml_dtypes.bfloat16)
    # static -k*y_m term of the bias, tiled per slot
    ky_np = np.tile(KSC * yg, NSLOT).reshape(P, 1).astype(np.float32)

    lhsT_d = nc.inline_tensor(lhsT_np, name="lhsT_main")
    ky_d = nc.inline_tensor(ky_np, name="ky_col")

    with tile.TileContext(nc) as tc, ExitStack() as ctx:
        singles = ctx.enter_context(tc.tile_pool(name="singles", bufs=1))
        nat = ctx.enter_context(tc.tile_pool(name="nat", bufs=2))
        small = ctx.enter_context(tc.tile_pool(name="small", bufs=2))
        splits = ctx.enter_context(tc.tile_pool(name="splits", bufs=1))
        rows = ctx.enter_context(tc.tile_pool(name="rows", bufs=3))
        hp = ctx.enter_context(tc.tile_pool(name="hp", bufs=1))
        ps_pool = ctx.enter_context(tc.tile_pool(name="ps", bufs=2, space="PSUM"))

        # queue roles: sync = input loads then reshape stream (in-order, loads
        # enqueued first); gpsimd = consts + final output (SWDGE, separate
        # resource from the global HWDGE the sync-queue DMAs serialize on).
        lhsT_sb = singles.tile([NSLOT, P], bf16)
        nc.sync.dma_start(out=lhsT_sb, in_=lhsT_d[:, :])
        ky_sb = singles.tile([P, 1], f32)
        nc.sync.dma_start(out=ky_sb, in_=ky_d[:, :])
        # preload the DErf activation table off the critical path
        dummy = singles.tile([1, 1], f32)
        nc.scalar.activation(
            out=dummy, in_=ky_sb[0:1, 0:1], func=ACTF.Derivative_Erf,
            bias=ky_sb[0:1, 0:1], scale=1.0,
        )

        scale_col = hp.tile([P, 1], f32, name="scale_col")
        bias_col = hp.tile([P, 1], f32, name="bias_col")

        zbs = []
        for u in range(4):
            p, t = divmod(u, 2)
            src_d = pred_d if t == 0 else targ_d
            z = nat.tile([P, F], f32, tag=f"z{u}")
            # loads split across the sync and scalar queues
            ldq = nc.sync if u < 2 else nc.scalar
            ldq.dma_start(out=z, in_=src_d[p, :].rearrange("(p f) -> p f", p=P))
            # cross-partition min/max: per-partition (-min, max) on DVE, one
            # GPSIMD all-reduce(max) -> [-gmin, gmax] on every partition
            mm = small.tile([P, 2], f32, tag=f"mm{u}")
            nc.vector.tensor_reduce(
                out=mm[:, 0:1], in_=z, axis=AX.X, op=OP.min, negate=True
            )
            nc.vector.tensor_reduce(out=mm[:, 1:2], in_=z, axis=AX.X, op=OP.max)
            # raw z in bf16 feeds the PE (quantization verified offline)
            zb = splits.tile([P, F], bf16, tag=f"zb{u}")
            nc.vector.tensor_copy(zb, z)
            zbs.append(zb)
            mmr = small.tile([P, 2], f32, tag=f"mmr{u}")
            nc.gpsimd.partition_all_reduce(mmr, mm, P, bass_isa.ReduceOp.max)
            # s = 1/(gmax - gmin + eps); ACT arg = (k s) z - k (s gmin + y_m)
            r = small.tile([P, 1], f32, tag=f"r{u}")
            nc.vector.tensor_scalar(r, mmr[:, 1:2], mmr[:, 0:1], EPS, OP.add, OP.add)
            s = small.tile([P, 1], f32, tag=f"s{u}")
            nc.vector.reciprocal(s, r)
            blk = slice(32 * u, 32 * u + 32)
            nc.vector.tensor_scalar_mul(scale_col[blk, :], s[blk, :], KSC)
            # t1 = s * (-gmin); bias = k*t1 - k*y_m
            t1 = small.tile([P, 1], f32, tag=f"t1{u}")
            nc.vector.tensor_mul(t1[blk, :], s[blk, :], mmr[blk, 0:1])
            nc.vector.tensor_scalar(
                bias_col[blk, :], t1[blk, :], KSC, ky_sb[blk, 0:1], OP.mult, OP.subtract
            )

        nblk = CHUNK // (ACTB * MMN)  # ACT blocks per chunk
        hparts = hp.tile([P, NCHUNK * nblk], f32, name="hparts")
        pp = CHUNK // F  # natural partitions per (chunk, half) slice (16)
        for ch in range(NCHUNK):
            rt = rows.tile([NSLOT, CHUNK], bf16, tag="rt")
            for u in range(4):
                for h in range(2):
                    p0 = 64 * h + pp * ch
                    # chunk0's DMAs split over two queues so the HWDGE
                    # serialization doesn't delay the stream start
                    q = nc.scalar if (ch == 0 and h == 1) else nc.sync
                    q.dma_start(
                        out=rt[2 * u + h : 2 * u + h + 1, :],
                        in_=zbs[u][p0 : p0 + pp, :],
                    )
            for hb in range(nblk):
                ps = ps_pool.tile([P, ACTB * MMN], f32, tag="ps")
                for i in range(ACTB):
                    c0 = hb * ACTB * MMN + i * MMN
                    nc.tensor.matmul(
                        ps[:, i * MMN : (i + 1) * MMN],
                        lhsT_sb[:, :],
                        rt[:, c0 : c0 + MMN],
                        start=True,
                        stop=True,
                    )
                icol = ch * nblk + hb
                nc.scalar.activation(
                    out=ps,
                    in_=ps,
                    func=ACTF.Derivative_Erf,
                    bias=bias_col[:, 0:1],
                    scale=scale_col[:, 0:1],
                    accum_out=hparts[:, icol : icol + 1],
                )

        uvec = small.tile([P, 1], f32, tag="uvec")
        nc.vector.tensor_reduce(out=uvec, in_=hparts, axis=AX.X, op=OP.add)
        nc.sync.dma_start(out=out_d[0, :], in_=uvec[:, 0:1])

    nc.compile()
    return nc


def kernel(pred: np.ndarray, target: np.ndarray) -> np.ndarray:
    from concourse.bass_utils import run_bass_kernel_spmd

    if "nc" not in _CACHE:
        _CACHE["nc"] = _build_nc()
        _CACHE["A"] = _interp_matrix()
    nc = _CACHE["nc"]
    A = _CACHE["A"]

    pred = np.ascontiguousarray(np.asarray(pred, np.float32).reshape(B, N))
    target = np.ascontiguousarray(np.asarray(target, np.float32).reshape(B, N))
    in_maps = [
        {
            "pred": pred[i * SPC : (i + 1) * SPC],
            "target": target[i * SPC : (i + 1) * SPC],
        }
        for i in range(NCORES)
    ]
    res = run_bass_kernel_spmd(nc, in_maps, list(range(NCORES)))
    us = np.stack([r["u_out"][0] for r in res.results], axis=0)  # [8, 128]
    us = us.astype(np.float64).reshape(NCORES, 4, 2, M)  # [core, unit, half, M]
    u = us.sum(axis=2).reshape(NCORES * 2, 2, M)  # [16 samples, pred/targ, M]
    # the DErf 2/sqrt(pi) constant cancels in the cdf normalization
    h = u @ A.T + EPS
    cdf = np.cumsum(h / h.sum(axis=-1, keepdims=True), axis=-1)
    return np.float32(np.mean((cdf[:, 0] - cdf[:, 1]) ** 2))


# revision 20
# speedup vs baseline: 2.6610x; 1.0302x over previous
"""Trainium2 Bass kernel for nn_CDFVarianceLoss.

Math (per sample b, per tensor z in {pred[b], target[b]}, N = 65536):
    z' = (z - min z) / (max z - min z + 1e-6)
    h_j = sum_n exp(-(z'_n - c_j)^2 / (2*sigma^2)) + 1e-6,  c_j = j/63, j < 64
    cdf = cumsum(h / sum_j h)
    loss = mean_{b,j} (cdf_pred[b,j] - cdf_target[b,j])^2

Algorithmic core: the 64-bin soft histogram is a Gaussian KDE, so the
device only samples the KDE u_m = sum_n exp(-alpha (z'_n - y_m)^2) on a
coarse M=16 grid y (the KDE spectrum is sigma-limited, so a fixed
least-squares matrix A reconstructs all 64 bins: h ~= A u, ~1e-6 relative
loss error verified offline; bf16-quantized z adds ~7e-5).  This cuts the
O(N*BINS) ACT work 4x vs dense 64-bin evaluation.

Evaluation trick: Derivative_Erf(x) = 2/sqrt(pi) exp(-x^2), so the whole
Gaussian evaluation needs only a LINEAR PSUM input z (no z^2 / no bf16
hi-lo splits): ACT computes DErf(scale_p * z + bias_p) where the
per-partition scale_p = k s_u and bias_p = -k (s_u zmin_u + y_m), k =
sqrt(alpha), fold the minmax normalization in for free (the 2/sqrt(pi)
constant cancels in the cdf normalization).  The PE matmul is a one-hot
[8, 128] bf16 broadcast: partition 16 s + m holds raw z of slot s, so
each matmul column carries 8 elements (one per slot = (sample, tensor,
column-half)) and the core needs only 32768 ACT/PE columns total.

Distribution: data-parallel over the batch (2 samples/core), 8 cores;
device returns the raw KDE samples u [128] per core; the host applies A,
eps, normalize/cumsum/diff/square/mean in fp64 (A is 64x16; trivial
flops, same spirit as the baseline's host-side mean).
"""

import numpy as np

B = 16
N = 65536
BINS = 64
SIGMA = 0.05
EPS = 1e-6
ALPHA = 0.5 / SIGMA**2  # 200.0
KSC = float(np.sqrt(ALPHA))  # DErf arg scale
NCORES = 8
SPC = B // NCORES  # samples per core
P = 128
F = N // P  # 512 natural free dim
M = 16  # KDE grid points per (sample, tensor) unit
RPAD = 0.05  # grid span padding beyond [0, 1]
NSLOT = 8  # 4 units x 2 column halves
COLS = N // 2  # matmul columns per core (each col = 8 elements)
CHUNK = 8192  # columns per reshape tile
NCHUNK = COLS // CHUNK  # 4
MMN = 512  # matmul moving free dim
ACTB = 4  # matmuls per ACT block (PSUM tile = 4 banks)

_CACHE = {}


def _grid():
    return np.linspace(-RPAD, 1.0 + RPAD, M)


def _interp_matrix():
    """Least-squares fit: h(c_j) ~= sum_m A[j,m] u(y_m) for any empirical
    distribution of z in [0,1] (the KDE's spectrum is sigma-limited, so the
    coarse grid determines it; verified offline to ~1e-6 loss error)."""
    yg = _grid()
    c = np.linspace(0.0, 1.0, BINS)
    zt = np.linspace(0.0, 1.0, 8001)
    Bm = np.exp(-ALPHA * (zt[:, None] - yg[None, :]) ** 2)  # [T, M]
    G = np.exp(-ALPHA * (c[:, None] - zt[None, :]) ** 2)  # [BINS, T]
    A = np.linalg.solve(Bm.T @ Bm + 1e-9 * np.eye(M), Bm.T @ G.T).T
    return A  # [BINS, M] float64


def _build_nc():
    import concourse.bass as bass
    import concourse.bacc as bacc
    import concourse.tile as tile
    import ml_dtypes
    from concourse import mybir
    from concourse import bass_isa
    from contextlib import ExitStack

    f32 = mybir.dt.float32
    bf16 = mybir.dt.bfloat16
    AX = mybir.AxisListType
    OP = mybir.AluOpType
    ACTF = mybir.ActivationFunctionType

    nc = bacc.Bacc()
    pred_d = nc.declare_dram_parameter("pred", [SPC, N], f32, isOutput=False)
    targ_d = nc.declare_dram_parameter("target", [SPC, N], f32, isOutput=False)
    out_d = nc.declare_dram_parameter("u_out", [1, P], f32, isOutput=True)

    yg = _grid().astype(np.float32)
    # one-hot broadcast: slot s's z row -> partitions 16s..16s+16; replicated
    # at partition offsets 0/32/64/96 so rotating PE tile positions can each
    # load their stationary from the matching SBUF start partition
    lhsT_np = np.zeros((P, P), np.float32)
    for g in range(4):
        for s in range(NSLOT):
            lhsT_np[32 * g + s, M * s : M * s + M] = 1.0
    lhsT_np = lhsT_np.astype(ml_dtypes.bfloat16)
    # static -k*y_m term of the bias, tiled per slot
    ky_np = np.tile(KSC * yg, NSLOT).reshape(P, 1).astype(np.float32)

    lhsT_d = nc.inline_tensor(lhsT_np, name="lhsT_main")
    ky_d = nc.inline_tensor(ky_np, name="ky_col")

    with tile.TileContext(nc) as tc, ExitStack() as ctx:
        singles = ctx.enter_context(tc.tile_pool(name="singles", bufs=1))
        nat = ctx.enter_context(tc.tile_pool(name="nat", bufs=2))
        small = ctx.enter_context(tc.tile_pool(name="small", bufs=2))
        splits = ctx.enter_context(tc.tile_pool(name="splits", bufs=1))
        rows = ctx.enter_context(tc.tile_pool(name="rows", bufs=3))
        hp = ctx.enter_context(tc.tile_pool(name="hp", bufs=1))
        ps_pool = ctx.enter_context(tc.tile_pool(name="ps", bufs=2, space="PSUM"))

        # queue roles: sync + scalar carry the two input loads then the
        # reshape stream; gpsimd (SWDGE, a separate resource from the global
        # HWDGE that sync/scalar DMAs serialize on) carries consts + output.
        lhsT_sb = singles.tile([P, P], bf16)
        nc.gpsimd.dma_start(out=lhsT_sb, in_=lhsT_d[:, :])
        ky_sb = singles.tile([P, 1], f32)
        nc.gpsimd.dma_start(out=ky_sb, in_=ky_d[:, :])
        # preload the DErf activation table off the critical path
        dummy = singles.tile([1, 1], f32)
        nc.scalar.activation(
            out=dummy, in_=ky_sb[0:1, 0:1], func=ACTF.Derivative_Erf,
            bias=ky_sb[0:1, 0:1], scale=1.0,
        )

        scale_col = hp.tile([P, 1], f32, name="scale_col")
        bias_col = hp.tile([P, 1], f32, name="bias_col")

        # one load per input tensor: both samples side by side in the free
        # dim ([128, (s f)]), DRAM side reordered by the descriptor walk
        zt_tiles = []
        zb_tiles = []
        for t, src_d in enumerate((pred_d, targ_d)):
            z = nat.tile([P, SPC * F], f32, tag=f"z{t}")
            ldq = nc.sync if t == 0 else nc.scalar
            ldq.dma_start(out=z, in_=src_d[:, :].rearrange("s (p f) -> p s f", p=P))
            zt_tiles.append(z)
            zb = splits.tile([P, SPC * F], bf16, tag=f"zb{t}")
            nc.vector.tensor_copy(zb, z)
            zb_tiles.append(zb)

        for u in range(4):
            p, t = divmod(u, 2)
            z = zt_tiles[t][:, p * F : (p + 1) * F]
            # cross-partition min/max: per-partition (-min, max) on DVE, one
            # GPSIMD all-reduce(max) -> [-gmin, gmax] on every partition
            mm = small.tile([P, 2], f32, tag=f"mm{u}")
            nc.vector.tensor_reduce(
                out=mm[:, 0:1], in_=z, axis=AX.X, op=OP.min, negate=True
            )
            nc.vector.tensor_reduce(out=mm[:, 1:2], in_=z, axis=AX.X, op=OP.max)
            mmr = small.tile([P, 2], f32, tag=f"mmr{u}")
            nc.gpsimd.partition_all_reduce(mmr, mm, P, bass_isa.ReduceOp.max)
            # s = 1/(gmax - gmin + eps); ACT arg = (k s) z - k (s gmin + y_m)
            r = small.tile([P, 1], f32, tag=f"r{u}")
            nc.vector.tensor_scalar(r, mmr[:, 1:2], mmr[:, 0:1], EPS, OP.add, OP.add)
            s = small.tile([P, 1], f32, tag=f"s{u}")
            nc.vector.reciprocal(s, r)
            blk = slice(32 * u, 32 * u + 32)
            nc.vector.tensor_scalar_mul(scale_col[blk, :], s[blk, :], KSC)
            # t1 = s * (-gmin); bias = k*t1 - k*y_m
            t1 = small.tile([P, 1], f32, tag=f"t1{u}")
            nc.vector.tensor_mul(t1[blk, :], s[blk, :], mmr[blk, 0:1])
            nc.vector.tensor_scalar(
                bias_col[blk, :], t1[blk, :], KSC, ky_sb[blk, 0:1], OP.mult, OP.subtract
            )

        nblk = CHUNK // (ACTB * MMN)  # ACT blocks per chunk
        hparts = hp.tile([P, NCHUNK * nblk], f32, name="hparts")
        pp = CHUNK // F  # natural partitions per (chunk, half) slice (16)
        for ch in range(NCHUNK):
            rt = rows.tile([NSLOT, CHUNK], bf16, tag="rt")
            for u in range(4):
                p, t = divmod(u, 2)
                for h in range(2):
                    p0 = 64 * h + pp * ch
                    # chunk0's DMAs split over two queues so the HWDGE
                    # serialization doesn't delay the stream start
                    q = nc.scalar if (ch == 0 and h == 1) else nc.sync
                    q.dma_start(
                        out=rt[2 * u + h : 2 * u + h + 1, :],
                        in_=zb_tiles[t][p0 : p0 + pp, p * F : (p + 1) * F],
                    )
            for hb in range(nblk):
                ps = ps_pool.tile([P, ACTB * MMN], f32, tag="ps")
                for i in range(ACTB):
                    c0 = hb * ACTB * MMN + i * MMN
                    nc.tensor.matmul(
                        ps[:, i * MMN : (i + 1) * MMN],
                        lhsT_sb[0:NSLOT, :],
                        rt[:, c0 : c0 + MMN],
                        start=True,
                        stop=True,
                    )
                icol = ch * nblk + hb
                nc.scalar.activation(
                    out=ps,
                    in_=ps,
                    func=ACTF.Derivative_Erf,
                    bias=bias_col[:, 0:1],
                    scale=scale_col[:, 0:1],
                    accum_out=hparts[:, icol : icol + 1],
                )

        uvec = small.tile([P, 1], f32, tag="uvec")
        nc.vector.tensor_reduce(out=uvec, in_=hparts, axis=AX.X, op=OP.add)
        nc.sync.dma_start(out=out_d[0, :], in_=uvec[:, 0:1])

    nc.compile()
    return nc


def kernel(pred: np.ndarray, target: np.ndarray) -> np.ndarray:
    from concourse.bass_utils import run_bass_kernel_spmd

    if "nc" not in _CACHE:
        _CACHE["nc"] = _build_nc()
        _CACHE["A"] = _interp_matrix()
    nc = _CACHE["nc"]
    A = _CACHE["A"]

    pred = np.ascontiguousarray(np.asarray(pred, np.float32).reshape(B, N))
    target = np.ascontiguousarray(np.asarray(target, np.float32).reshape(B, N))
    in_maps = [
        {
            "pred": pred[i * SPC : (i + 1) * SPC],
            "target": target[i * SPC : (i + 1) * SPC],
        }
        for i in range(NCORES)
    ]
    res = run_bass_kernel_spmd(nc, in_maps, list(range(NCORES)))
    us = np.stack([r["u_out"][0] for r in res.results], axis=0)  # [8, 128]
    us = us.astype(np.float64).reshape(NCORES, 4, 2, M)  # [core, unit, half, M]
    u = us.sum(axis=2).reshape(NCORES * 2, 2, M)  # [16 samples, pred/targ, M]
    # the DErf 2/sqrt(pi) constant cancels in the cdf normalization
    h = u @ A.T + EPS
    cdf = np.cumsum(h / h.sum(axis=-1, keepdims=True), axis=-1)
    return np.float32(np.mean((cdf[:, 0] - cdf[:, 1]) ** 2))


# revision 21
# speedup vs baseline: 2.7355x; 1.0280x over previous
"""Trainium2 Bass kernel for nn_CDFVarianceLoss.

Math (per sample b, per tensor z in {pred[b], target[b]}, N = 65536):
    z' = (z - min z) / (max z - min z + 1e-6)
    h_j = sum_n exp(-(z'_n - c_j)^2 / (2*sigma^2)) + 1e-6,  c_j = j/63, j < 64
    cdf = cumsum(h / sum_j h)
    loss = mean_{b,j} (cdf_pred[b,j] - cdf_target[b,j])^2

Algorithmic core: the 64-bin soft histogram is a Gaussian KDE, so the
device only samples the KDE u_m = sum_n exp(-alpha (z'_n - y_m)^2) on a
coarse M=16 grid y (the KDE spectrum is sigma-limited, so a fixed
least-squares matrix A reconstructs all 64 bins: h ~= A u, ~1e-6 relative
loss error verified offline; bf16-quantized z adds ~7e-5).  This cuts the
O(N*BINS) ACT work 4x vs dense 64-bin evaluation.

Evaluation trick: Derivative_Erf(x) = 2/sqrt(pi) exp(-x^2), so the whole
Gaussian evaluation needs only a LINEAR PSUM input z (no z^2 / no bf16
hi-lo splits): ACT computes DErf(scale_p * z + bias_p) where the
per-partition scale_p = k s_u and bias_p = -k (s_u zmin_u + y_m), k =
sqrt(alpha), fold the minmax normalization in for free (the 2/sqrt(pi)
constant cancels in the cdf normalization).  The PE matmul is a one-hot
[8, 128] bf16 broadcast: partition 16 s + m holds raw z of slot s, so
each matmul column carries 8 elements (one per slot = (sample, tensor,
column-half)) and the core needs only 32768 ACT/PE columns total.

Distribution: data-parallel over the batch (2 samples/core), 8 cores;
device returns the raw KDE samples u [128] per core; the host applies A,
eps, normalize/cumsum/diff/square/mean in fp64 (A is 64x16; trivial
flops, same spirit as the baseline's host-side mean).
"""

import numpy as np

B = 16
N = 65536
BINS = 64
SIGMA = 0.05
EPS = 1e-6
ALPHA = 0.5 / SIGMA**2  # 200.0
KSC = float(np.sqrt(ALPHA))  # DErf arg scale
NCORES = 8
SPC = B // NCORES  # samples per core
P = 128
F = N // P  # 512 natural free dim
M = 16  # KDE grid points per (sample, tensor) unit
RPAD = 0.05  # grid span padding beyond [0, 1]
NSLOT = 8  # 4 units x 2 column halves
COLS = N // 2  # matmul columns per core (each col = 8 elements)
CHUNK = 8192  # columns per reshape tile
NCHUNK = COLS // CHUNK  # 4
MMN = 512  # matmul moving free dim
ACTB = 4  # matmuls per ACT block (PSUM tile = 4 banks)

_CACHE = {}


def _grid():
    return np.linspace(-RPAD, 1.0 + RPAD, M)


def _interp_matrix():
    """Least-squares fit: h(c_j) ~= sum_m A[j,m] u(y_m) for any empirical
    distribution of z in [0,1] (the KDE's spectrum is sigma-limited, so the
    coarse grid determines it; verified offline to ~1e-6 loss error)."""
    yg = _grid()
    c = np.linspace(0.0, 1.0, BINS)
    zt = np.linspace(0.0, 1.0, 8001)
    Bm = np.exp(-ALPHA * (zt[:, None] - yg[None, :]) ** 2)  # [T, M]
    G = np.exp(-ALPHA * (c[:, None] - zt[None, :]) ** 2)  # [BINS, T]
    A = np.linalg.solve(Bm.T @ Bm + 1e-9 * np.eye(M), Bm.T @ G.T).T
    return A  # [BINS, M] float64


def _build_nc():
    import concourse.bass as bass
    import concourse.bacc as bacc
    import concourse.tile as tile
    import ml_dtypes
    from concourse import mybir
    from concourse import bass_isa
    from contextlib import ExitStack

    f32 = mybir.dt.float32
    bf16 = mybir.dt.bfloat16
    AX = mybir.AxisListType
    OP = mybir.AluOpType
    ACTF = mybir.ActivationFunctionType

    nc = bacc.Bacc()
    pred_d = nc.declare_dram_parameter("pred", [SPC, N], f32, isOutput=False)
    targ_d = nc.declare_dram_parameter("target", [SPC, N], f32, isOutput=False)
    out_d = nc.declare_dram_parameter("u_out", [1, P], f32, isOutput=True)

    yg = _grid().astype(np.float32)
    # one-hot broadcast: slot s's z row -> partitions 16s..16s+16; replicated
    # at partition offsets 0/32/64/96 so rotating PE tile positions can each
    # load their stationary from the matching SBUF start partition
    lhsT_np = np.zeros((P, P), np.float32)
    for g in range(4):
        for s in range(NSLOT):
            lhsT_np[32 * g + s, M * s : M * s + M] = 1.0
    lhsT_np = lhsT_np.astype(ml_dtypes.bfloat16)
    # static -k*y_m term of the bias, tiled per slot
    ky_np = np.tile(KSC * yg, NSLOT).reshape(P, 1).astype(np.float32)

    lhsT_d = nc.inline_tensor(lhsT_np, name="lhsT_main")
    ky_d = nc.inline_tensor(ky_np, name="ky_col")

    with tile.TileContext(nc) as tc, ExitStack() as ctx:
        singles = ctx.enter_context(tc.tile_pool(name="singles", bufs=1))
        nat = ctx.enter_context(tc.tile_pool(name="nat", bufs=2))
        small = ctx.enter_context(tc.tile_pool(name="small", bufs=2))
        splits = ctx.enter_context(tc.tile_pool(name="splits", bufs=1))
        rows = ctx.enter_context(tc.tile_pool(name="rows", bufs=3))
        hp = ctx.enter_context(tc.tile_pool(name="hp", bufs=1))
        ps_pool = ctx.enter_context(tc.tile_pool(name="ps", bufs=2, space="PSUM"))

        # queue roles: sync + scalar carry the two input loads then the
        # reshape stream; gpsimd (SWDGE, a separate resource from the global
        # HWDGE that sync/scalar DMAs serialize on) carries consts + output.
        lhsT_sb = singles.tile([P, P], bf16)
        nc.gpsimd.dma_start(out=lhsT_sb, in_=lhsT_d[:, :])
        ky_sb = singles.tile([P, 1], f32)
        nc.gpsimd.dma_start(out=ky_sb, in_=ky_d[:, :])
        # preload the DErf activation table off the critical path
        dummy = singles.tile([1, 1], f32)
        nc.scalar.activation(
            out=dummy, in_=ky_sb[0:1, 0:1], func=ACTF.Derivative_Erf,
            bias=ky_sb[0:1, 0:1], scale=1.0,
        )

        scale_col = hp.tile([P, 1], f32, name="scale_col")
        bias_col = hp.tile([P, 1], f32, name="bias_col")

        # one load per input tensor: both samples side by side in the free
        # dim ([128, (s f)]), DRAM side reordered by the descriptor walk
        zt_tiles = []
        zb_tiles = []
        for t, src_d in enumerate((pred_d, targ_d)):
            z = nat.tile([P, SPC * F], f32, tag=f"z{t}")
            ldq = nc.sync if t == 0 else nc.scalar
            ldq.dma_start(out=z, in_=src_d[:, :].rearrange("s (p f) -> p s f", p=P))
            zt_tiles.append(z)
            zb = splits.tile([P, SPC * F], bf16, tag=f"zb{t}")
            nc.vector.tensor_copy(zb, z)
            zb_tiles.append(zb)

        # cross-partition min/max, all 4 units batched: per-partition
        # (-min, max) pairs in one [128, 8] tile, one GPSIMD all-reduce(max),
        # then vectorized [128, 4] assembly of the ACT scale/bias columns
        mm = small.tile([P, 8], f32, name="mm_all")
        for u in range(4):
            p, t = divmod(u, 2)
            z = zt_tiles[t][:, p * F : (p + 1) * F]
            nc.vector.tensor_reduce(
                out=mm[:, 2 * u : 2 * u + 1], in_=z, axis=AX.X, op=OP.min, negate=True
            )
            nc.vector.tensor_reduce(
                out=mm[:, 2 * u + 1 : 2 * u + 2], in_=z, axis=AX.X, op=OP.max
            )
        mmr = small.tile([P, 8], f32, name="mmr_all")
        nc.gpsimd.partition_all_reduce(mmr, mm, P, bass_isa.ReduceOp.max)
        neg_min = mmr[:, :].rearrange("p (u c) -> p u c", c=2)[:, :, 0]  # [P, 4]
        gmax = mmr[:, :].rearrange("p (u c) -> p u c", c=2)[:, :, 1]
        r4 = small.tile([P, 4], f32, name="r4")
        nc.vector.tensor_tensor(out=r4, in0=gmax, in1=neg_min, op=OP.add)
        nc.vector.tensor_scalar_add(r4, r4, EPS)
        s4 = small.tile([P, 4], f32, name="s4")
        nc.vector.reciprocal(s4, r4)
        t14 = small.tile([P, 4], f32, name="t14")
        nc.vector.tensor_mul(t14, s4, neg_min)  # s * (-gmin)
        for u in range(4):
            blk = slice(32 * u, 32 * u + 32)
            nc.vector.tensor_scalar_mul(scale_col[blk, :], s4[blk, u : u + 1], KSC)
            nc.vector.tensor_scalar(
                bias_col[blk, :], t14[blk, u : u + 1], KSC, ky_sb[blk, 0:1],
                OP.mult, OP.subtract,
            )

        nblk = CHUNK // (ACTB * MMN)  # ACT blocks per chunk
        hparts = hp.tile([P, NCHUNK * nblk], f32, name="hparts")
        pp = CHUNK // F  # natural partitions per (chunk, half) slice (16)
        for ch in range(NCHUNK):
            rt = rows.tile([NSLOT, CHUNK], bf16, tag="rt")
            for u in range(4):
                p, t = divmod(u, 2)
                for h in range(2):
                    p0 = 64 * h + pp * ch
                    # chunk0's DMAs split over two queues so the HWDGE
                    # serialization doesn't delay the stream start
                    q = nc.scalar if (ch == 0 and h == 1) else nc.sync
                    q.dma_start(
                        out=rt[2 * u + h : 2 * u + h + 1, :],
                        in_=zb_tiles[t][p0 : p0 + pp, p * F : (p + 1) * F],
                    )
            for hb in range(nblk):
                ps = ps_pool.tile([P, ACTB * MMN], f32, tag="ps")
                for i in range(ACTB):
                    c0 = hb * ACTB * MMN + i * MMN
                    nc.tensor.matmul(
                        ps[:, i * MMN : (i + 1) * MMN],
                        lhsT_sb[0:NSLOT, :],
                        rt[:, c0 : c0 + MMN],
                        start=True,
                        stop=True,
                    )
                icol = ch * nblk + hb
                nc.scalar.activation(
                    out=ps,
                    in_=ps,
                    func=ACTF.Derivative_Erf,
                    bias=bias_col[:, 0:1],
                    scale=scale_col[:, 0:1],
                    accum_out=hparts[:, icol : icol + 1],
                )

        uvec = small.tile([P, 1], f32, tag="uvec")
        nc.vector.tensor_reduce(out=uvec, in_=hparts, axis=AX.X, op=OP.add)
        nc.sync.dma_start(out=out_d[0, :], in_=uvec[:, 0:1])

    nc.compile()
    return nc


def kernel(pred: np.ndarray, target: np.ndarray) -> np.ndarray:
    from concourse.bass_utils import run_bass_kernel_spmd

    if "nc" not in _CACHE:
        _CACHE["nc"] = _build_nc()
        _CACHE["A"] = _interp_matrix()
    nc = _CACHE["nc"]
    A = _CACHE["A"]

    pred = np.ascontiguousarray(np.asarray(pred, np.float32).reshape(B, N))
    target = np.ascontiguousarray(np.asarray(target, np.float32).reshape(B, N))
    in_maps = [
        {
            "pred": pred[i * SPC : (i + 1) * SPC],
            "target": target[i * SPC : (i + 1) * SPC],
        }
        for i in range(NCORES)
    ]
    res = run_bass_kernel_spmd(nc, in_maps, list(range(NCORES)))
    us = np.stack([r["u_out"][0] for r in res.results], axis=0)  # [8, 128]
    us = us.astype(np.float64).reshape(NCORES, 4, 2, M)  # [core, unit, half, M]
    u = us.sum(axis=2).reshape(NCORES * 2, 2, M)  # [16 samples, pred/targ, M]
    # the DErf 2/sqrt(pi) constant cancels in the cdf normalization
    h = u @ A.T + EPS
    cdf = np.cumsum(h / h.sum(axis=-1, keepdims=True), axis=-1)
    return np.float32(np.mean((cdf[:, 0] - cdf[:, 1]) ** 2))
